# revision 16
# baseline (speedup 1.0000x reference)
"""Trainium2 Bass kernel for LocalFeatureAggregation (KNN + 3 fused MLPs + max-pool).

Sharding: 8 cores = 4 batches x 2 query-halves. Each core processes NQ=4096
query points against all N=8192 candidates of its batch.

Math notes (host-folded):
  - eval-BN folded into conv weights: W' = W * g/sqrt(v+eps), b' = b - m*s.
  - MLP on [ctr, nbr-ctr] splits: W'@[c, n-c] = (Wa-Wb)@c + Wb@n
    -> per-query term G/U (computed once per query) and per-candidate term
       H/V (computed once per candidate point), fused y = relu(G+H | U+V).
  - distance: d(q,j) = |p_q|^2 + |p_j|^2 - 2<p_q,p_j>; PE computes
    [-2x_q,-2y_q,-2z_q,1] . [x_j,y_j,z_j,|p_j|^2] = d - |p_q|^2, ACT adds
    |p_q|^2 and negates so DVE max8 selects the 16 nearest.
"""

import os

import numpy as np

B, N, C, O, K = 4, 8192, 64, 64, 16
EPS = 1e-5
P = 128
NEG_BIG = -3.0e38

_BUILD_CACHE = {}

# GU injection through PE (identity-matmul preload + transpose accumulate).
# Falls back to Pool-engine add+relu if disabled.
USE_PE_INJECT = os.environ.get("KNN_PE_INJECT", "1") == "1"


def build(n_tiles=32):
    """Build the single-core Bass module processing n_tiles*128 queries."""
    key = (n_tiles, USE_PE_INJECT)
    if key in _BUILD_CACHE:
        return _BUILD_CACHE[key]

    import concourse.bacc as bacc
    import concourse.mybir as mybir
    from concourse import tile
    from concourse.bass import IndirectOffsetOnAxis
    from concourse.masks import make_identity

    f32 = mybir.dt.float32
    u32 = mybir.dt.uint32
    Act = mybir.ActivationFunctionType
    Alu = mybir.AluOpType
    NQ = n_tiles * P

    nc = bacc.Bacc("TRN2", target_bir_lowering=False, debug=False)

    # ---- I/O ----
    AQ = nc.dram_tensor("AQ", (3, NQ), f32, kind="ExternalInput")    # [2x;2y;2z] at query cols
    R = nc.dram_tensor("R", (3, N), f32, kind="ExternalInput")       # [x;y;z] all candidates
    SQJN = nc.dram_tensor("SQJN", (1, N), f32, kind="ExternalInput")  # -|p_j|^2 row
    FT = nc.dram_tensor("FT", (C, N), f32, kind="ExternalInput")     # features^T all candidates
    FQ = nc.dram_tensor("FQ", (C, NQ), f32, kind="ExternalInput")    # features^T at query cols
    NSQ = nc.dram_tensor("NSQ", (P, n_tiles), f32, kind="ExternalInput")  # -|p_q|^2
    W1N = nc.dram_tensor("W1N", (3, O), f32, kind="ExternalInput")   # W1nbr^T
    W1C = nc.dram_tensor("W1C", (3, O), f32, kind="ExternalInput")   # (-0.5*W1ctr)^T
    W2N = nc.dram_tensor("W2N", (C, O), f32, kind="ExternalInput")   # W2nbr^T
    W2C = nc.dram_tensor("W2C", (C, O), f32, kind="ExternalInput")   # W2ctr^T
    W3T = nc.dram_tensor("W3T", (2 * O, O), f32, kind="ExternalInput")
    B1 = nc.dram_tensor("B1", (O, 1), f32, kind="ExternalInput")
    B2 = nc.dram_tensor("B2", (O, 1), f32, kind="ExternalInput")
    B3 = nc.dram_tensor("B3", (O, 1), f32, kind="ExternalInput")
    OUT = nc.dram_tensor("OUT", (NQ, O), f32, kind="ExternalOutput")

    with tile.TileContext(nc) as tc:
        with (
            tc.tile_pool(name="const", bufs=1) as cpool,
            tc.tile_pool(name="dram", bufs=1, space="DRAM") as dpool,
        ):
            hvt_dram = dpool.tile([N, 2 * O], f32)

            ident = cpool.tile([P, P], f32)
            make_identity(nc, ident[:])

            aq = cpool.tile([3, NQ], f32)
            nc.sync.dma_start(aq[:], AQ[:])
            r = cpool.tile([3, N], f32)
            nc.sync.dma_start(r[:], R[:])
            # -|p_j|^2 replicated across all 128 partitions (for exact
            # fl(-sq_j - sq_q) matching the reference's association)
            sqjb = cpool.tile([P, N], f32)
            nc.sync.dma_start(sqjb[0:1, :], SQJN[:])
            nc.gpsimd.partition_broadcast(sqjb[:], sqjb[0:1, :])
            nsq = cpool.tile([P, n_tiles], f32)
            nc.sync.dma_start(nsq[:], NSQ[:])
            w1n = cpool.tile([3, O], f32)
            nc.sync.dma_start(w1n[:], W1N[:])
            w1c = cpool.tile([3, O], f32)
            nc.sync.dma_start(w1c[:], W1C[:])
            w2n = cpool.tile([C, O], f32)
            nc.sync.dma_start(w2n[:], W2N[:])
            w2c = cpool.tile([C, O], f32)
            nc.sync.dma_start(w2c[:], W2C[:])
            w3t = cpool.tile([2 * O, O], f32)
            nc.sync.dma_start(w3t[:], W3T[:])
            b1 = cpool.tile([O, 1], f32)
            nc.sync.dma_start(b1[:], B1[:])
            b2 = cpool.tile([O, 1], f32)
            nc.sync.dma_start(b2[:], B2[:])
            b3 = cpool.tile([O, 1], f32)
            nc.sync.dma_start(b3[:], B3[:])

            gu = cpool.tile([2 * O, NQ], f32)  # [G;U] channel-major per query

            # ================= prep: H/V -> hvt_dram, G/U -> gu =================
            with (
                tc.tile_pool(name="prep_ps", bufs=2, space="PSUM") as pps,
                tc.tile_pool(name="prep_sb", bufs=3) as psb,
                tc.tile_pool(name="prep_big", bufs=1) as pbig,
            ):
                ft = pbig.tile([C, N], f32)
                nc.sync.dma_start(ft[:], FT[:])
                fq = pbig.tile([C, NQ], f32)
                nc.sync.dma_start(fq[:], FQ[:])
                # H/V chunk (64, 512) -> transpose 4x(64,128) -> hvt_dram
                for c in range(N // 512):
                    sl = slice(c * 512, (c + 1) * 512)
                    for which, wT, rhs_ap, col0 in (
                        ("h", w1n, r[:, sl], 0),
                        ("v", w2n, ft[:, sl], O),
                    ):
                        hp = pps.tile([O, 512], f32, tag="hp")
                        nc.tensor.matmul(hp[:], lhsT=wT[:], rhs=rhs_ap, start=True, stop=True)
                        hc = psb.tile([O, 512], f32, tag="hc")
                        nc.scalar.copy(hc[:], hp[:])
                        for j in range(4):
                            rs = slice(c * 512 + j * P, c * 512 + (j + 1) * P)
                            tp = pps.tile([P, O], f32, tag="tp")
                            nc.tensor.transpose(tp[:], hc[:, j * P:(j + 1) * P], ident[0:O, 0:O])
                            ts = psb.tile([P, O], f32, tag="ts")
                            nc.scalar.copy(ts[:], tp[:])
                            nc.sync.dma_start(hvt_dram[rs, col0:col0 + O], ts[:])
                # G -> gu[0:64]; U -> u_tmp -> DMA -> gu[64:128]
                u_tmp = pbig.tile([O, NQ], f32)
                GC = min(512, NQ)
                for c in range(NQ // GC):
                    sl = slice(c * GC, (c + 1) * GC)
                    gp = pps.tile([O, GC], f32, tag="gp")
                    nc.tensor.matmul(gp[:], lhsT=w1c[:], rhs=aq[:, sl], start=True, stop=True)
                    nc.scalar.activation(gu[0:O, sl], gp[:], Act.Identity, bias=b1[:, 0:1])
                    up = pps.tile([O, GC], f32, tag="gp")
                    nc.tensor.matmul(up[:], lhsT=w2c[:], rhs=fq[:, sl], start=True, stop=True)
                    nc.scalar.activation(u_tmp[:, sl], up[:], Act.Identity, bias=b2[:, 0:1])
                nc.sync.dma_start(gu[O:2 * O, :], u_tmp[:])

            # ================= main loop over query tiles =================
            with (
                tc.tile_pool(name="dps", bufs=2, space="PSUM") as dps,
                tc.tile_pool(name="fps", bufs=2, space="PSUM") as fps,
                tc.tile_pool(name="yps", bufs=2, space="PSUM") as yps,
                tc.tile_pool(name="ops", bufs=2, space="PSUM") as ops_,
                tc.tile_pool(name="dbuf", bufs=2) as dbuf,
                tc.tile_pool(name="small", bufs=2) as spool,
                tc.tile_pool(name="gtp", bufs=1) as gtp,
                tc.tile_pool(name="fup", bufs=2) as fup,
            ):
                for t in range(n_tiles):
                    qsl = slice(t * P, (t + 1) * P)
                    # ---- distances (negated): D[q, j] = -(d(q,j)) ----
                    # psum = 2<p_q, p_j>; D = fl(fl(-sq_j - sq_q) + 2dot)
                    # = -fl(fl(sq_q + sq_j) - 2dot), bit-matching the reference.
                    D = dbuf.tile([P, N], f32, tag="D")
                    for c in range(N // 512):
                        sl = slice(c * 512, (c + 1) * 512)
                        dp = dps.tile([P, 512], f32, tag="dp")
                        nc.tensor.matmul(dp[:], lhsT=aq[:, qsl], rhs=r[:, sl],
                                         start=True, stop=True)
                        pdot = spool.tile([P, 512], f32, tag="pdot")
                        nc.scalar.copy(pdot[:], dp[:])
                        ssum = spool.tile([P, 512], f32, tag="ssum")
                        nc.scalar.activation(ssum[:], sqjb[:, sl], Act.Identity,
                                             bias=nsq[:, t:t + 1])
                        nc.gpsimd.tensor_tensor(out=D[:, sl], in0=ssum[:],
                                                in1=pdot[:], op=Alu.add)
                    # ---- top-16 (largest of -d) with indices ----
                    vals_a = spool.tile([P, 8], f32, tag="va")
                    vals_b = spool.tile([P, 8], f32, tag="vb")
                    idx = spool.tile([P, K], u32, tag="idx")
                    nc.vector.max(vals_a[:], D[:])
                    nc.vector.max_index(idx[:, 0:8], vals_a[:], D[:])
                    nc.vector.match_replace(D[:], vals_a[:], D[:], NEG_BIG)
                    nc.vector.max(vals_b[:], D[:])
                    nc.vector.max_index(idx[:, 8:16], vals_b[:], D[:])
                    # ---- gather [H;V] rows for each neighbor ----
                    gt = gtp.tile([P, K, 2 * O], f32, tag="gt")
                    for k in range(K):
                        nc.gpsimd.indirect_dma_start(
                            out=gt[:, k, :], out_offset=None,
                            in_=hvt_dram[:],
                            in_offset=IndirectOffsetOnAxis(ap=idx[:, k:k + 1], axis=0),
                        )
                    # ---- fused = relu(gt^T + GU) channel-major ----
                    fused = fup.tile([P, K * P], f32, tag="fused")
                    for k in range(K):
                        fp = fps.tile([P, P], f32, tag="fp")
                        if USE_PE_INJECT:
                            nc.tensor.matmul(fp[:], lhsT=ident[:], rhs=gu[:, qsl],
                                             start=True, stop=False)
                            nc.tensor.matmul(fp[:], lhsT=gt[:, k, :], rhs=ident[:],
                                             start=False, stop=True, is_transpose=True)
                            nc.scalar.activation(fused[:, k * P:(k + 1) * P], fp[:], Act.Relu)
                        else:
                            nc.tensor.transpose(fp[:], gt[:, k, :], ident[:])
                            fsl = fused[:, k * P:(k + 1) * P]
                            nc.gpsimd.tensor_tensor(out=fsl, in0=fp[:], in1=gu[:, qsl], op=Alu.add)
                            nc.gpsimd.tensor_scalar_max(fsl, fsl, 0.0)
                    # ---- y3 = W3' @ fused; max over k; +b3; relu ----
                    reds = []
                    for c4 in range(4):
                        yp = yps.tile([O, 512], f32, tag="yp")
                        nc.tensor.matmul(yp[:], lhsT=w3t[:],
                                         rhs=fused[:, c4 * 512:(c4 + 1) * 512],
                                         start=True, stop=True)
                        red = spool.tile([O, P], f32, tag=f"red{c4}")
                        nc.vector.tensor_reduce(
                            red[:], yp[:].rearrange("o (k q) -> o q k", k=4),
                            op=Alu.max, axis=mybir.AxisListType.X)
                        reds.append(red)
                    m = spool.tile([O, P], f32, tag="m")
                    nc.vector.tensor_tensor(out=m[:], in0=reds[0][:], in1=reds[1][:], op=Alu.max)
                    nc.vector.tensor_tensor(out=m[:], in0=m[:], in1=reds[2][:], op=Alu.max)
                    nc.vector.tensor_tensor(out=m[:], in0=m[:], in1=reds[3][:], op=Alu.max)
                    mb = spool.tile([O, P], f32, tag="mb")
                    # (m + b3) then relu
                    nc.vector.tensor_scalar(mb[:], m[:], b3[:, 0:1], scalar2=0.0,
                                            op0=Alu.add, op1=Alu.max)
                    # ---- transpose (O, P) -> (P, O) and store ----
                    ot = ops_.tile([P, O], f32, tag="ot")
                    nc.tensor.transpose(ot[:], mb[:], ident[0:O, 0:O])
                    osb = spool.tile([P, O], f32, tag="osb")
                    nc.scalar.copy(osb[:], ot[:])
                    nc.sync.dma_start(OUT[qsl, :], osb[:])

    nc.compile()
    _BUILD_CACHE[key] = nc
    return nc


def host_prep(points, features, w_geom, g1, b1, m1, v1,
              w_sem, g2, b2, m2, v2, w_fuse, g3, b3, m3, v3,
              n_tiles=32):
    """Fold BN into weights, build per-core input maps."""
    NQ = n_tiles * P
    pts = np.asarray(points, np.float32)
    feats = np.asarray(features, np.float32)
    x, y, z = pts[..., 0], pts[..., 1], pts[..., 2]
    # match jnp.sum(p*p, -1) rounding: fl(fl(x^2 + y^2) + z^2)
    sq = ((x * x + y * y) + z * z).astype(np.float32)         # (B, N)
    A_all = np.stack([2 * x, 2 * y, 2 * z], axis=1)           # (B, 3, N)
    R_all = np.stack([x, y, z], axis=1)                       # (B, 3, N)
    FT_all = np.ascontiguousarray(feats.transpose(0, 2, 1))   # (B, C, N)

    def fold(w, g, b, m, v):
        s = np.asarray(g, np.float32) / np.sqrt(np.asarray(v, np.float32) + EPS)
        return np.asarray(w, np.float32) * s[:, None], (np.asarray(b, np.float32) - np.asarray(m, np.float32) * s)

    W1, b1f = fold(w_geom, g1, b1, m1, v1)
    W2, b2f = fold(w_sem, g2, b2, m2, v2)
    W3, b3f = fold(w_fuse, g3, b3, m3, v3)

    W1N = np.ascontiguousarray(W1[:, 3:].T)                     # (3, O)
    W1C = np.ascontiguousarray((0.5 * (W1[:, :3] - W1[:, 3:])).T)   # (3, O), rhs = +2p
    W2N = np.ascontiguousarray(W2[:, C:].T)                     # (C, O)
    W2C = np.ascontiguousarray((W2[:, :C] - W2[:, C:]).T)       # (C, O)
    W3T = np.ascontiguousarray(W3.T)                            # (2O, O)

    in_maps = []
    for core in range(8):
        b_, h = core // 2, core % 2
        qsl = slice(h * NQ, (h + 1) * NQ)
        nsq_core = -sq[b_, qsl].reshape(n_tiles, P).T  # (P, n_tiles): [p, t] = -sq[t*128+p]
        in_maps.append({
            "AQ": np.ascontiguousarray(A_all[b_][:, qsl]),
            "R": np.ascontiguousarray(R_all[b_]),
            "SQJN": np.ascontiguousarray(-sq[b_]).reshape(1, N),
            "FT": FT_all[b_],
            "FQ": np.ascontiguousarray(FT_all[b_][:, qsl]),
            "NSQ": np.ascontiguousarray(nsq_core),
            "W1N": W1N, "W1C": W1C, "W2N": W2N, "W2C": W2C, "W3T": W3T,
            "B1": b1f.reshape(O, 1), "B2": b2f.reshape(O, 1), "B3": b3f.reshape(O, 1),
        })
    return in_maps


def kernel(**inputs):
    from concourse.bass_utils import run_bass_kernel_spmd

    n_tiles = (N // 2) // P
    nc = build(n_tiles)
    in_maps = host_prep(**inputs, n_tiles=n_tiles)
    res = run_bass_kernel_spmd(nc, in_maps, core_ids=list(range(8)))
    NQ = n_tiles * P
    out = np.empty((B, N, O), np.float32)
    for core in range(8):
        b_, h = core // 2, core % 2
        out[b_, h * NQ:(h + 1) * NQ, :] = res.results[core]["OUT"]
    return out
